# revision 1
# baseline (speedup 1.0000x reference)
"""Trainium2 Bass kernel: 3x3 stride-1 pad-1 Conv2D, NCHW.

Problem: x (32,128,56,56) f32, weight (256,128,3,3) OIHW, bias (256,)
-> out (32,256,56,56) f32.

Strategy: data-parallel over batch N across 8 NeuronCores (4 images per
core), weights/bias replicated. Per core: implicit GEMM — C_in=128 is
exactly the SBUF partition dim; for each of the 9 filter taps we issue a
128x128 (ci x co-chunk) matmul against a shifted window of the
host-padded image, accumulating all 9 taps into one PSUM bank. fp16
matmul (1 cycle/row) gives ~4x over plain fp32 at ~2.7e-4 rel err.
"""

import numpy as np

import concourse.bass as bass
import concourse.mybir as mybir
import concourse.tile as tile
from concourse import bacc
from concourse.bass_utils import run_bass_kernel_spmd

N_CORES = 8
N_FULL = 32
N_PER_CORE = N_FULL // N_CORES  # 4
CIN = 128
COUT = 256
H = W = 56
HP = WP = 58  # padded spatial
R = 8  # output rows per matmul tile
NT = H // R  # 7 row-tiles per image
NFREE = R * W  # 448 (<= 512 PSUM-bank limit per matmul)
F32 = mybir.dt.float32
F32R = mybir.dt.float32r
F16 = mybir.dt.float16

# Module-level knobs for the dev harness (test.py). The grading harness
# just calls kernel(**inputs) and gets the default (no-trace) path.
TRACE = False
LAST_RESULT = None

_prog = None


def _build_program():
    nc = bacc.Bacc("TRN2", target_bir_lowering=False, debug=False)
    x_d = nc.declare_dram_parameter("x", [N_PER_CORE, CIN, HP * WP], F16, isOutput=False)
    w_d = nc.declare_dram_parameter("wt", [CIN, 9 * COUT], F16, isOutput=False)
    b_d = nc.declare_dram_parameter("bias", [COUT], F32, isOutput=False)
    # fp16 output (|y| <~ 2, rel err 2^-11): halves store traffic + teardown
    # fence latency vs f32; host upcasts.
    out_d = nc.declare_dram_parameter(
        "out", [N_PER_CORE, 2, 128, H * W], F16, isOutput=True
    )

    CH = (R + 2) * WP  # one chunk: R output rows + 2 halo rows of padded input

    with tile.TileContext(nc) as tc:
        with (
            tc.tile_pool(name="const", bufs=1) as const_pool,
            tc.tile_pool(name="xin", bufs=4) as x_pool,
            tc.tile_pool(name="outp", bufs=4) as out_pool,
            tc.tile_pool(name="psum", bufs=7, space="PSUM") as psum_pool,
        ):
            # Critical-path startup loads are split HALF/HALF across both
            # HWDGE queues (sync + scalar) so the first compute group's data
            # (x chunk 0 + all of w c=0) lands ~1.5us sooner after the ~7us
            # engine-start barrier than a single-queue w load would.
            w_sbs = []
            for c in range(2):
                w_c = const_pool.tile([CIN, 9 * 128], F16, tag=f"w{c}")
                w_sbs.append(w_c)
            bias_sb = const_pool.tile([128, 2], F32)

            # Warmup: dummy matmuls fill the PE during the initial DMA wait, so
            # HAM un-throttles (needs ~3.4us of sustained PE activity) before
            # the first real matmul. Memset on gpsimd (idle at startup) so the
            # busy vector/sync/scalar engines don't gate the warm matmuls.
            scratch = const_pool.tile([128, NFREE], F16)
            nc.gpsimd.memset(scratch[:], 0.0)
            warm_ps = psum_pool.tile([128, NFREE], F32, tag="warm", bufs=1)
            NWARM = 8
            for wi in range(NWARM):
                nc.tensor.matmul(
                    warm_ps[:], lhsT=scratch[:, :128], rhs=scratch[:],
                    start=(wi == 0), stop=(wi == NWARM - 1), skip_group_check=True,
                )

            # Per-image, per-row-block input chunks (overlapping halo rows) so
            # the first matmuls only wait on a ~300KB DMA, not whole images.
            x_view = x_d[:].rearrange("n p (h w) -> n p h w", w=WP)
            x_tiles = {}

            def load_chunk(i, r, split=False):
                x_c = x_pool.tile([CIN, CH], F16)
                if split:
                    half = (R + 2) // 2
                    for eng, lo, hi in (
                        (nc.sync, 0, half),
                        (nc.scalar, half, R + 2),
                    ):
                        eng.dma_start(
                            out=x_c[:, lo * WP : hi * WP],
                            in_=x_view[i][:, r * R + lo : r * R + hi, :],
                        )
                else:
                    nc.scalar.dma_start(
                        out=x_c[:],
                        in_=x_view[i][:, r * R : r * R + R + 2, :],
                    )
                x_tiles[(i, r)] = x_c

            def compute_tile(i, c, r, row0=0, nrows=R, store_eng=None):
                x_img = x_tiles[(i, r)][:].rearrange("p (h w) -> p h w", w=WP)
                nf = nrows * W
                psum_t = psum_pool.tile([128, NFREE], F32)
                psum_v = psum_t[:, :nf].rearrange("p (r w) -> p r w", w=W)
                for k in range(9):
                    kh, kw = divmod(k, 3)
                    rhs = x_img[:, row0 + kh : row0 + kh + nrows, kw : kw + W]
                    lhsT = w_sbs[c][:, k * 128 : (k + 1) * 128]
                    nc.tensor.matmul(
                        psum_v, lhsT=lhsT, rhs=rhs, start=(k == 0), stop=(k == 8)
                    )
                out_t = out_pool.tile([128, NFREE], F16)
                nc.vector.tensor_scalar_add(
                    out_t[:, :nf], psum_t[:, :nf], bias_sb[:, c : c + 1]
                )
                lo = r * NFREE + row0 * W
                (store_eng or nc.sync).dma_start(
                    out=out_d[i, c][:, lo : lo + nf], in_=out_t[:, :nf]
                )

            # Emission order = DMA queue order. Startup schedule (each queue
            # ~120 GB/s, dispatch unblocks ~7us): x0 halves (74KB each), then
            # w c=0 halves (213KB each) -> first group ready ~9.4us; then
            # bias + w c=1 halves -> ready just as group (0,0,c=1) starts.
            load_chunk(0, 0, split=True)
            HW9 = 9 * 128 // 2
            for c in range(2):
                if c == 1:
                    # Bias is tiny but DMAs as 256 4-byte descriptors; emit it
                    # after the c=0 weights (first needed at the first
                    # copy-out), before the later c=1 weights.
                    for cc in range(2):
                        nc.scalar.dma_start(
                            out=bias_sb[:, cc : cc + 1],
                            in_=b_d[cc * 128 : (cc + 1) * 128].rearrange(
                                "(p one) -> p one", one=1
                            ),
                        )
                for eng, lo, hi in ((nc.sync, 0, HW9), (nc.scalar, HW9, 9 * 128)):
                    eng.dma_start(
                        out=w_sbs[c][:, lo:hi],
                        in_=w_d[:, c * 9 * 128 + lo : c * 9 * 128 + hi],
                    )
            load_chunk(0, 1)
            for i in range(N_PER_CORE):
                for r in range(NT):
                    nxt = (i, r + 2) if r + 2 < NT else (i + 1, (r + 2) % NT)
                    if nxt[0] < N_PER_CORE and nxt not in x_tiles:
                        load_chunk(*nxt)
                    last = i == N_PER_CORE - 1 and r == NT - 1
                    compute_tile(i, 0, r)
                    if last:
                        # Shorten the tail: the final copy-out + store chain
                        # handles 4 rows instead of 8.
                        compute_tile(i, 1, r, row0=0, nrows=4)
                        compute_tile(i, 1, r, row0=4, nrows=4)
                    else:
                        compute_tile(i, 1, r)
                    del x_tiles[(i, r)]
    nc.compile()
    return nc


def kernel(x: np.ndarray, weight: np.ndarray, bias: np.ndarray) -> np.ndarray:
    global _prog, LAST_RESULT
    x = np.ascontiguousarray(x, dtype=np.float32)
    weight = np.ascontiguousarray(weight, dtype=np.float32)
    bias = np.ascontiguousarray(bias, dtype=np.float32)

    # Host-side prep: pad spatial dims, shard batch, pre-transpose weights.
    x_pad = np.zeros((N_FULL, CIN, HP, WP), dtype=np.float16)
    x_pad[:, :, 1:-1, 1:-1] = x
    x_pad = x_pad.reshape(N_FULL, CIN, HP * WP)

    # wt[ci, (c*9 + k)*128 + co2] = weight[c*128 + co2, ci, kh, kw], k = kh*3+kw
    # (c-major so the c=0 half is one contiguous DMA)
    wt = np.ascontiguousarray(
        weight.reshape(2, 128, CIN, 9).transpose(2, 0, 3, 1).reshape(CIN, 9 * COUT)
    ).astype(np.float16)

    if _prog is None:
        _prog = _build_program()

    in_maps = [
        {
            "x": np.ascontiguousarray(x_pad[i * N_PER_CORE : (i + 1) * N_PER_CORE]),
            "wt": wt,
            "bias": bias,
        }
        for i in range(N_CORES)
    ]
    res = run_bass_kernel_spmd(_prog, in_maps, list(range(N_CORES)), trace=TRACE)
    LAST_RESULT = res
    out = np.concatenate([r["out"] for r in res.results], axis=0)
    return out.astype(np.float32).reshape(N_FULL, COUT, H, W)



# revision 3
# speedup vs baseline: 1.1598x; 1.1598x over previous
"""Trainium2 Bass kernel: 3x3 stride-1 pad-1 Conv2D, NCHW, via 1D Winograd.

Problem: x (32,128,56,56) f32, weight (256,128,3,3) OIHW, bias (256,)
-> out (32,256,56,56) f32.

Strategy: data-parallel over batch N across 8 NeuronCores (4 images per
core), weights/bias replicated. Per core: Winograd F(2,3) along H —
output rows are produced in pairs; the 3 vertical taps collapse into 4
"pos" products shared by both rows of a pair (2x row reuse), cutting PE
streaming cycles 1.5x vs the direct 9-tap implicit GEMM:

  t[pos]    = B^T d        (row combos of the input, DVE fp16)
  m[pos]    = sum_kw Gg[kw,pos]^T @ t[pos](shifted kw)   (PE, PSUM acc.)
  out pair  = A^T m:  o0 = m0+m1+m2,  o1 = m1-m2-m3

The A^T combine is spread across engines so it all hides under the PE:
ACT copies m1,m2 PSUM->SBUF, GPSIMD forms s=m1+m2 / d=m1-m2, DVE fuses
the final add with the remaining PSUM read via scalar_tensor_tensor.
Weight transform Gg = G @ W_taps is folded on the host; bias (zeros in
this problem, but handled generally) is added on the host after gather.
"""

import numpy as np

import concourse.bass as bass
import concourse.mybir as mybir
import concourse.tile as tile
from concourse import bacc
from concourse.bass_utils import run_bass_kernel_spmd

N_CORES = 8
N_FULL = 32
N_PER_CORE = N_FULL // N_CORES  # 4
CIN = 128
COUT = 256
H = W = 56
HP = WP = 58  # padded spatial
NPAIR = H // 2  # 28 row-pairs per image
QB = 7  # row-pairs per block
NB = NPAIR // QB  # 4 blocks per image
NF = QB * W  # 392 matmul free dim (pairs x width)
ROWS = 2 * QB + 2  # 16 padded input rows per block (14 + 2 halo)
F32 = mybir.dt.float32
F16 = mybir.dt.float16

# Module-level knobs for the dev harness (test.py). The grading harness
# just calls kernel(**inputs) and gets the default (no-trace) path.
TRACE = False
LAST_RESULT = None

_prog = None


def _build_program():
    nc = bacc.Bacc("TRN2", target_bir_lowering=False, debug=False)
    x_d = nc.declare_dram_parameter("x", [N_PER_CORE, CIN, HP * WP], F16, isOutput=False)
    # wt[ci, ((c*4 + pos)*3 + kw)*128 + co2] = Gg, host-transformed
    w_d = nc.declare_dram_parameter("wt", [CIN, 24 * 128], F16, isOutput=False)
    out_d = nc.declare_dram_parameter(
        "out", [N_PER_CORE, 2, 128, H * W], F16, isOutput=True
    )

    AluOp = mybir.AluOpType
    ActFn = mybir.ActivationFunctionType

    with tile.TileContext(nc) as tc:
        with (
            tc.tile_pool(name="const", bufs=1) as const_pool,
            tc.tile_pool(name="xin", bufs=4) as x_pool,
            tc.tile_pool(name="tin", bufs=3) as t_pool,
            tc.tile_pool(name="mc", bufs=3) as mc_pool,
            tc.tile_pool(name="sd", bufs=3) as sd_pool,
            tc.tile_pool(name="outp", bufs=4) as out_pool,
            tc.tile_pool(name="psum", bufs=8, space="PSUM") as psum_pool,
        ):
            w_sbs = []
            for c in range(2):
                w_c = const_pool.tile([CIN, 12 * 128], F16, tag=f"w{c}")
                w_sbs.append(w_c)

            # Warmup: dummy matmuls fill the PE during the initial DMA wait so
            # HAM un-throttles before the first real matmul. Memset on gpsimd
            # (idle at startup).
            scratch = const_pool.tile([128, NF], F16)
            nc.gpsimd.memset(scratch[:], 0.0)
            warm_ps = psum_pool.tile([128, NF], F32, tag="ps")
            NWARM = 8
            for wi in range(NWARM):
                nc.tensor.matmul(
                    warm_ps[:], lhsT=scratch[:, :128], rhs=scratch[:],
                    start=(wi == 0), stop=(wi == NWARM - 1), skip_group_check=True,
                )

            x_view = x_d[:].rearrange("n p (h w) -> n p h w", w=WP)
            x_tiles = {}

            def load_chunk(i, b, split=False):
                # Padded input rows [14b, 14b+16) of image i.
                x_c = x_pool.tile([CIN, ROWS, WP], F16)
                r0 = b * 2 * QB
                if split:
                    half = ROWS // 2
                    for eng, lo, hi in ((nc.sync, 0, half), (nc.scalar, half, ROWS)):
                        eng.dma_start(
                            out=x_c[:, lo:hi, :],
                            in_=x_view[i][:, r0 + lo : r0 + hi, :],
                        )
                else:
                    nc.sync.dma_start(
                        out=x_c[:], in_=x_view[i][:, r0 : r0 + ROWS, :]
                    )
                x_tiles[(i, b)] = x_c

            # Emission order = DMA queue order. Startup: x block (0,0) halves
            # first (both queues), then w c=0 halves, then w c=1, then the
            # next x block.
            load_chunk(0, 0, split=True)
            HW = 12 * 128 // 2
            for c in range(2):
                for eng, lo, hi in ((nc.sync, 0, HW), (nc.scalar, HW, 12 * 128)):
                    eng.dma_start(
                        out=w_sbs[c][:, lo:hi],
                        in_=w_d[:, c * 12 * 128 + lo : c * 12 * 128 + hi],
                    )
            load_chunk(0, 1)

            # A^T combine pos order: m1, m2 first so their PSUM banks close
            # (and ACT copies start) while the PE still streams m0, m3.
            POS_ORDER = (1, 2, 0, 3)

            def compute_block(i, b):
                xt = x_tiles[(i, b)]
                # Input transform t[pos] = B^T d over row pairs (fp16, DVE):
                #   t0 = d0-d2, t1 = d1+d2, t2 = d2-d1, t3 = d1-d3
                t_t = t_pool.tile([CIN, 4, QB, WP], F16)
                E = 2 * QB  # 14
                for pos, (a0, a1, op) in enumerate((
                    (0, 2, AluOp.subtract),
                    (1, 2, AluOp.add),
                    (2, 1, AluOp.subtract),
                    (1, 3, AluOp.subtract),
                )):
                    nc.vector.tensor_tensor(
                        t_t[:, pos],
                        xt[:, a0 : a0 + E - 1 : 2, :],
                        xt[:, a1 : a1 + E - 1 : 2, :],
                        op,
                    )
                for c in range(2):
                    ms = {}
                    for pos in POS_ORDER:
                        ps = psum_pool.tile([128, NF], F32, tag="ps")
                        ps_v = ps[:].rearrange("p (q w) -> p q w", w=W)
                        for kw in range(3):
                            lhsT = w_sbs[c][:, ((pos * 3 + kw) * 128):((pos * 3 + kw + 1) * 128)]
                            nc.tensor.matmul(
                                ps_v, lhsT=lhsT, rhs=t_t[:, pos, :, kw : kw + W],
                                start=(kw == 0), stop=(kw == 2),
                            )
                        ms[pos] = ps
                    # Drain: o0 = m0 + (m1+m2), o1 = (-m3) + (m1-m2)
                    mc = mc_pool.tile([128, 2, NF], F32)
                    nc.scalar.activation(mc[:, 0], ms[1][:], ActFn.Copy)
                    nc.scalar.activation(mc[:, 1], ms[2][:], ActFn.Copy)
                    s_t = sd_pool.tile([128, NF], F32, tag="s")
                    d_t = sd_pool.tile([128, NF], F32, tag="d")
                    nc.gpsimd.tensor_tensor(s_t[:], mc[:, 0], mc[:, 1], AluOp.add)
                    nc.gpsimd.tensor_tensor(d_t[:], mc[:, 0], mc[:, 1], AluOp.subtract)
                    out_t = out_pool.tile([128, QB, 2, W], F16)
                    nc.vector.scalar_tensor_tensor(
                        out_t[:, :, 0, :],
                        ms[0][:].rearrange("p (q w) -> p q w", w=W),
                        0.0,
                        s_t[:].rearrange("p (q w) -> p q w", w=W),
                        AluOp.bypass,
                        AluOp.add,
                    )
                    nc.vector.scalar_tensor_tensor(
                        out_t[:, :, 1, :],
                        ms[3][:].rearrange("p (q w) -> p q w", w=W),
                        -1.0,
                        d_t[:].rearrange("p (q w) -> p q w", w=W),
                        AluOp.mult,
                        AluOp.add,
                    )
                    lo = b * 2 * QB * W
                    store_eng = nc.sync if c == 0 else nc.scalar
                    store_eng.dma_start(
                        out=out_d[i, c][:, lo : lo + 2 * NF],
                        in_=out_t[:].rearrange("p q j w -> p (q j w)"),
                    )

            for i in range(N_PER_CORE):
                for b in range(NB):
                    nxt = (i, b + 2) if b + 2 < NB else (i + 1, (b + 2) % NB)
                    if nxt[0] < N_PER_CORE and nxt not in x_tiles:
                        load_chunk(*nxt)
                    compute_block(i, b)
                    del x_tiles[(i, b)]
    nc.compile()
    return nc


# F(2,3) weight transform matrix (applied over the kh axis).
_G = np.array(
    [[1.0, 0.0, 0.0], [0.5, 0.5, 0.5], [0.5, -0.5, 0.5], [0.0, 0.0, 1.0]],
    dtype=np.float64,
)


def kernel(x: np.ndarray, weight: np.ndarray, bias: np.ndarray) -> np.ndarray:
    global _prog, LAST_RESULT
    x = np.ascontiguousarray(x, dtype=np.float32)
    weight = np.ascontiguousarray(weight, dtype=np.float32)
    bias = np.ascontiguousarray(bias, dtype=np.float32)

    # Host-side prep: pad spatial dims, shard batch, Winograd-transform the
    # weights over kh: Gg[pos][co,ci,kw] = sum_kh G[pos,kh] W[co,ci,kh,kw].
    x_pad = np.zeros((N_FULL, CIN, HP, WP), dtype=np.float16)
    x_pad[:, :, 1:-1, 1:-1] = x
    x_pad = x_pad.reshape(N_FULL, CIN, HP * WP)

    u = np.einsum("ph,oihw->oipw", _G, weight.astype(np.float64))
    # wt[ci, ((c*4 + pos)*3 + kw)*128 + co2]
    wt = np.ascontiguousarray(
        u.reshape(2, 128, CIN, 4, 3).transpose(2, 0, 3, 4, 1).reshape(CIN, 24 * 128)
    ).astype(np.float16)

    if _prog is None:
        _prog = _build_program()

    in_maps = [
        {
            "x": np.ascontiguousarray(x_pad[i * N_PER_CORE : (i + 1) * N_PER_CORE]),
            "wt": wt,
        }
        for i in range(N_CORES)
    ]
    res = run_bass_kernel_spmd(_prog, in_maps, list(range(N_CORES)), trace=TRACE)
    LAST_RESULT = res
    out = np.concatenate([r["out"] for r in res.results], axis=0)
    out = out.astype(np.float32).reshape(N_FULL, COUT, H, W)
    if bias.any():
        out += bias[None, :, None, None]
    return out


# revision 5
# speedup vs baseline: 1.2421x; 1.0709x over previous
"""Trainium2 Bass kernel: 3x3 stride-1 pad-1 Conv2D, NCHW, via 1D Winograd.

Problem: x (32,128,56,56) f32, weight (256,128,3,3) OIHW, bias (256,)
-> out (32,256,56,56) f32.

Strategy: data-parallel over batch N across 8 NeuronCores (4 images per
core), weights/bias replicated. Per core: Winograd F(2,3) along H —
output rows are produced in pairs; the 3 vertical taps collapse into 4
"pos" products shared by both rows of a pair (2x row reuse), cutting PE
streaming cycles 1.5x vs the direct 9-tap implicit GEMM:

  t[pos]    = B^T d        (row combos of the input, DVE fp16, per image)
  m[pos]    = sum_kw Gg[kw,pos]^T @ t[pos](shifted kw)   (PE, PSUM acc.)
  out pair  = A^T m:  o0 = m0+m1+m2,  o1 = m1-m2-m3

The A^T combine is spread across engines so it all hides under the PE:
ACT copies m1,m2 PSUM->SBUF (fp16), GPSIMD forms s=m1+m2, DVE forms
d=m1-m2 and fuses the final adds with the remaining PSUM reads via
scalar_tensor_tensor. Weight transform Gg = G @ W_taps is folded on the
host; bias (zeros in this problem, but handled generally) is added on
the host after gather.
"""

import numpy as np

import concourse.bass as bass
import concourse.mybir as mybir
import concourse.tile as tile
from concourse import bacc
from concourse.bass_utils import run_bass_kernel_spmd

N_CORES = 8
N_FULL = 32
N_PER_CORE = N_FULL // N_CORES  # 4
CIN = 128
COUT = 256
H = W = 56
HP = WP = 58  # padded spatial
NPAIR = H // 2  # 28 row-pairs per image
QB = 7  # row-pairs per block
NB = NPAIR // QB  # 4 blocks per image
NF = QB * W  # 392 matmul free dim (pairs x width)
F32 = mybir.dt.float32
F16 = mybir.dt.float16

# Module-level knobs for the dev harness (test.py). The grading harness
# just calls kernel(**inputs) and gets the default (no-trace) path.
TRACE = False
LAST_RESULT = None

_prog = None


def _build_program():
    nc = bacc.Bacc("TRN2", target_bir_lowering=False, debug=False)
    x_d = nc.declare_dram_parameter("x", [N_PER_CORE, CIN, HP * WP], F16, isOutput=False)
    # wt[ci, ((c*4 + pos)*3 + kw)*128 + co2] = Gg, host-transformed
    w_d = nc.declare_dram_parameter("wt", [CIN, 24 * 128], F16, isOutput=False)
    out_d = nc.declare_dram_parameter(
        "out", [N_PER_CORE, 2, 128, H * W], F16, isOutput=True
    )

    AluOp = mybir.AluOpType
    ActFn = mybir.ActivationFunctionType

    # A^T combine pos order: m1, m2 first so their PSUM banks close (and
    # the drain chain starts) while the PE still streams m0, m3.
    POS_ORDER = (1, 2, 0, 3)
    # Input-transform row combos per pos: t[pos] = d[a0] op d[a1]
    T_DEFS = (
        (0, 2, AluOp.subtract),  # t0 = d0 - d2
        (1, 2, AluOp.add),       # t1 = d1 + d2
        (2, 1, AluOp.subtract),  # t2 = d2 - d1
        (1, 3, AluOp.subtract),  # t3 = d1 - d3
    )

    with tile.TileContext(nc) as tc:
        with (
            tc.tile_pool(name="const", bufs=1) as const_pool,
            tc.tile_pool(name="xin", bufs=2) as x_pool,
            tc.tile_pool(name="tin", bufs=2) as t_pool,
            tc.tile_pool(name="mc", bufs=3) as mc_pool,
            tc.tile_pool(name="sd", bufs=3) as sd_pool,
            tc.tile_pool(name="outp", bufs=4) as out_pool,
            tc.tile_pool(name="psum", bufs=8, space="PSUM") as psum_pool,
        ):
            w_sbs = []
            for c in range(2):
                w_c = const_pool.tile([CIN, 12 * 128], F16, tag=f"w{c}")
                w_sbs.append(w_c)

            x_view = x_d[:].rearrange("n p (h w) -> n p h w", w=WP)
            x_tiles = {}

            def load_image(i):
                x_c = x_pool.tile([CIN, HP, WP], F16)
                half = HP // 2
                for eng, lo, hi in ((nc.sync, 0, half), (nc.scalar, half, HP)):
                    eng.dma_start(out=x_c[:, lo:hi, :], in_=x_view[i][:, lo:hi, :])
                x_tiles[i] = x_c

            # Emission order = DMA queue order. Startup: x image 0 halves on
            # both queues, then w c=0 halves (needed by the first matmul
            # group), then w c=1, then image 1.
            load_image(0)
            HW = 12 * 128 // 2
            for c in range(2):
                for eng, lo, hi in ((nc.sync, 0, HW), (nc.scalar, HW, 12 * 128)):
                    eng.dma_start(
                        out=w_sbs[c][:, lo:hi],
                        in_=w_d[:, c * 12 * 128 + lo : c * 12 * 128 + hi],
                    )

            # Warmup: dummy matmuls on the (tiny-valued) weight tile fill the
            # PE during the initial DMA wait so HAM un-throttles before the
            # first real matmul. No memset needed - w c0 lands first.
            warm_ps = psum_pool.tile([128, NF], F32, tag="ps")
            NWARM = 8
            for wi in range(NWARM):
                nc.tensor.matmul(
                    warm_ps[:], lhsT=w_sbs[0][:, :128], rhs=w_sbs[0][:, :NF],
                    start=(wi == 0), stop=(wi == NWARM - 1), skip_group_check=True,
                )

            load_image(1)

            t_tiles = {}

            def transform_image(i):
                # t[pos] = B^T d over all 28 row pairs at once: even middle
                # dim (28) keeps the DVE 2x perf mode eligible.
                xt = x_tiles[i]
                t_t = t_pool.tile([CIN, 4, NPAIR, WP], F16)
                E = 2 * NPAIR - 1  # strided slice end: covers rows a..a+54
                for pos in POS_ORDER:
                    a0, a1, op = T_DEFS[pos]
                    nc.vector.tensor_tensor(
                        t_t[:, pos],
                        xt[:, a0 : a0 + E : 2, :],
                        xt[:, a1 : a1 + E : 2, :],
                        op,
                    )
                t_tiles[i] = t_t

            transform_image(0)

            store_ctr = [0]

            def compute_block(i, b, split_drain=False):
                t_t = t_tiles[i]
                for c in range(2):
                    ms = {}
                    for pos in POS_ORDER:
                        ps = psum_pool.tile([128, NF], F32, tag="ps")
                        ps_v = ps[:].rearrange("p (q w) -> p q w", w=W)
                        for kw in range(3):
                            lhsT = w_sbs[c][
                                :, ((pos * 3 + kw) * 128):((pos * 3 + kw + 1) * 128)
                            ]
                            nc.tensor.matmul(
                                ps_v,
                                lhsT=lhsT,
                                rhs=t_t[:, pos, b * QB : b * QB + QB, kw : kw + W],
                                start=(kw == 0), stop=(kw == 2),
                            )
                        ms[pos] = ps
                    # Drain: o0 = m0 + (m1+m2), o1 = (-m3) + (m1-m2)
                    out_t = out_pool.tile([128, QB, 2, W], F16)
                    qsplits = ((0, 4), (4, QB)) if split_drain else ((0, QB),)
                    for qn, (q0, q1) in enumerate(qsplits):
                        n = (q1 - q0) * W
                        sl = slice(q0 * W, q1 * W)
                        mc = mc_pool.tile([128, 2, NF], F16)
                        nc.scalar.activation(mc[:, 0, sl], ms[1][:, sl], ActFn.Copy)
                        nc.scalar.activation(mc[:, 1, sl], ms[2][:, sl], ActFn.Copy)
                        s_t = sd_pool.tile([128, NF], F16, tag="s")
                        d_t = sd_pool.tile([128, NF], F16, tag="d")
                        nc.gpsimd.tensor_tensor(
                            s_t[:, sl], mc[:, 0, sl], mc[:, 1, sl], AluOp.add
                        )
                        nc.vector.tensor_tensor(
                            d_t[:, sl], mc[:, 0, sl], mc[:, 1, sl], AluOp.subtract
                        )
                        for j, m_ps, sd_t, op0, scl in (
                            (0, ms[0], s_t, AluOp.bypass, 0.0),
                            (1, ms[3], d_t, AluOp.mult, -1.0),
                        ):
                            nc.vector.scalar_tensor_tensor(
                                out_t[:, q0:q1, j, :],
                                m_ps[:].rearrange("p (q w) -> p q w", w=W)[:, q0:q1],
                                scl,
                                sd_t[:].rearrange("p (q w) -> p q w", w=W)[:, q0:q1],
                                op0,
                                AluOp.add,
                            )
                        lo = (b * QB + q0) * 2 * W
                        store_eng = nc.sync if store_ctr[0] % 2 == 0 else nc.scalar
                        store_ctr[0] += 1
                        store_eng.dma_start(
                            out=out_d[i, c][:, lo : lo + (q1 - q0) * 2 * W],
                            in_=out_t[:, q0:q1].rearrange("p q j w -> p (q j w)"),
                        )

            for i in range(N_PER_CORE):
                if i + 2 < N_PER_CORE:
                    load_image(i + 2)
                for b in range(NB):
                    last = i == N_PER_CORE - 1 and b == NB - 1
                    if b == 0 and i + 1 < N_PER_CORE:
                        transform_image(i + 1)
                    compute_block(i, b, split_drain=last)
                del x_tiles[i], t_tiles[i]
    nc.compile()
    return nc


# F(2,3) weight transform matrix (applied over the kh axis).
_G = np.array(
    [[1.0, 0.0, 0.0], [0.5, 0.5, 0.5], [0.5, -0.5, 0.5], [0.0, 0.0, 1.0]],
    dtype=np.float64,
)


def kernel(x: np.ndarray, weight: np.ndarray, bias: np.ndarray) -> np.ndarray:
    global _prog, LAST_RESULT
    x = np.ascontiguousarray(x, dtype=np.float32)
    weight = np.ascontiguousarray(weight, dtype=np.float32)
    bias = np.ascontiguousarray(bias, dtype=np.float32)

    # Host-side prep: pad spatial dims, shard batch, Winograd-transform the
    # weights over kh: Gg[pos][co,ci,kw] = sum_kh G[pos,kh] W[co,ci,kh,kw].
    x_pad = np.zeros((N_FULL, CIN, HP, WP), dtype=np.float16)
    x_pad[:, :, 1:-1, 1:-1] = x
    x_pad = x_pad.reshape(N_FULL, CIN, HP * WP)

    u = np.einsum("ph,oihw->oipw", _G, weight.astype(np.float64))
    # wt[ci, ((c*4 + pos)*3 + kw)*128 + co2]
    wt = np.ascontiguousarray(
        u.reshape(2, 128, CIN, 4, 3).transpose(2, 0, 3, 4, 1).reshape(CIN, 24 * 128)
    ).astype(np.float16)

    if _prog is None:
        _prog = _build_program()

    in_maps = [
        {
            "x": np.ascontiguousarray(x_pad[i * N_PER_CORE : (i + 1) * N_PER_CORE]),
            "wt": wt,
        }
        for i in range(N_CORES)
    ]
    res = run_bass_kernel_spmd(_prog, in_maps, list(range(N_CORES)), trace=TRACE)
    LAST_RESULT = res
    out = np.concatenate([r["out"] for r in res.results], axis=0)
    out = out.astype(np.float32).reshape(N_FULL, COUT, H, W)
    if bias.any():
        out += bias[None, :, None, None]
    return out
